# revision 1
# baseline (speedup 1.0000x reference)
"""Trainium2 Bass kernel for nn_AttentionBasedMerger.

Reference computation (per batch element b, SQ=1):
  q = input @ Wq + bq                      -> (NH, HD)  [tiny]
  k = retrieval @ Wk + bk                  -> (SK, NH, HD)
  v = retrieval @ Wv + bv                  -> (SK, NH, HD)
  scores[h,j] = cos_sim(q[h], k[j,h])
  p = (scores+1)/2 ; 2-way gumbel-softmax gate with external uniform noise
  probs[h,j] = gate[...,0]
  ctx[h] = sum_j probs[h,j] v[j,h]         -> (NH, HD)
  out = ctx.flat @ Wd + bd                 -> (HID,)

Algebraic restructuring used here (exact up to fp reassociation):
  - v-projection is never materialized:
      ctx[h] = (sum_j probs[h,j] x[j]) @ Wv_h + (sum_j probs[h,j]) * bv_h
    so only a (NH x SK) @ (SK x HID) GEMM ("m"-matmul) plus a tiny
    per-head (HID x HD) GEMM is needed -- 64x fewer FLOPs than full v.
  - scores come from the k-projection run once:
      s_raw[j,h]  = x[j] @ (Wk @ qhat_blockdiag)  (extra 16 psum columns)
      kbk[j,h]    = x[j] @ wbk                    (bias correction columns)
      ssq[j,h]    = sum_d k0[j,h,d]^2             (squared-eviction + reduce)
      ||k||^2     = ssq + 2*kbk + sum(bk_h^2)
      scores      = (s_raw + qhat.bk_h) * rsqrt(||k||^2)
  - the 2-way gumbel softmax collapses to a stable rational:
      probs = p*A1 / (p*A1 + (1-p)*A0),  A_i = EPS - log(u_i + EPS)
    (A0/A1 are pure elementwise transforms of the noise input, applied on
    the host during input staging).

Sharding: pure data-parallel over batch, 8 batch elements per core.
"""

import os
import sys

sys.path.insert(0, "/opt/trn_rl_repo")

import numpy as np

import concourse.bass as bass
import concourse.tile as tile
from concourse import bacc, mybir
from concourse.bass_utils import run_bass_kernel_spmd
from concourse.masks import make_identity

F32 = mybir.dt.float32
F32R = mybir.dt.float32r
BF16 = mybir.dt.bfloat16
AX = mybir.AxisListType
OP = mybir.AluOpType
AF = mybir.ActivationFunctionType

B, SK, HID, NH, HD = 64, 2048, 1024, 16, 64
NCORES = 8
BL = B // NCORES  # 8 batch elems per core
CI = HID // 128  # 8 contraction chunks
JC = SK // 128  # 16 seq chunks
JG = 2  # seq chunks per xT load group
EPS = 1e-20

# matmul precision mode for the two large GEMM stages:
#   "f32"  - native fp32 matmul (4 cyc/row). HW-measured rel err 1.1e-6.
#   "f32r" - relaxed-precision fp32 PE mode (1 cyc/row at free dim >= 256)
#            for the k-projection; the m-matmul runs as bf16 hi+lo 3-pass
#            when biases are zero (second-order error, unlike f32r's
#            first-order rounding which measured 1.5e-4 there). HW-measured
#            scale-relative err 2.3e-5, ~2.9x faster than "f32".
#   "hilo" - bf16 hi+lo 3-pass decomposition (~fp32 class error).
MM_MODE = os.environ.get("MM_MODE", "f32r")


def build_nc(mode=MM_MODE, nobias=False):
    """nobias=True: all of bq/bk/bv/bd are exactly zero (checked on the host)
    -> drop the bias-correction psum columns and gate terms."""
    nc = bacc.Bacc("TRN2", target_bir_lowering=False, debug=False, num_devices=NCORES)

    dram = {}

    def din(name, shape, dt=F32):
        dram[name] = nc.dram_tensor(name, list(shape), dt, kind="ExternalInput").ap()
        return dram[name]

    u_a0 = din("a0", [BL, SK, NH])  # EPS - log(u0 + EPS)
    u_a1 = din("a1", [BL, SK, NH])
    inT = din("inT", [HID, BL])  # input_tensor slice, transposed
    wq = din("wq", [HID, HID])
    wv = din("wv", [HID, HID])
    wd = din("wd", [HID, HID])
    bq = din("bq", [1, HID])
    bk = din("bk", [1, HID])
    bv = din("bv", [1, HID])
    bd = din("bd", [1, HID])
    sbrep = din("sbrep", [1, NH * JC])  # tile(sum(bk_h^2), JC)
    wbk = din("wbk", [HID, NH])
    wkT = din("wkT", [HID, HID])  # Wk transposed (for wq_eff = Wk @ qhat_blk)
    if mode == "hilo":
        xt_h = din("xt_h", [BL, HID, SK], BF16)
        xt_l = din("xt_l", [BL, HID, SK], BF16)
        wk_h = din("wk_h", [HID, HID], BF16)
        wk_l = din("wk_l", [HID, HID], BF16)
        xn_h = din("xn_h", [BL, SK, HID], BF16)
        xn_l = din("xn_l", [BL, SK, HID], BF16)
        wk = None
        xt = None
        xn = None
    else:
        xdt = F32R if mode == "f32r" else F32
        xt = din("xt", [BL, HID, SK], xdt)  # retrieval slice, transposed
        wk = din("wk", [HID, HID], xdt)
        if mode == "f32r" and nobias:
            # bf16 hi+lo m-matmul: same bytes as one fp32 copy of xn
            xn = None
            xn_h = din("xn_h", [BL, SK, HID], BF16)
            xn_l = din("xn_l", [BL, SK, HID], BF16)
        else:
            xn = din("xn", [BL, SK, HID])  # natural retrieval slice

    out = nc.dram_tensor("o", [BL, HID], F32, kind="ExternalOutput").ap()

    xdt = F32R if mode == "f32r" else F32
    SW = 16 if nobias else 32  # s-psum columns: qhat (+ wbk correction)
    mhilo = mode == "hilo" or (mode == "f32r" and nobias)  # bf16 hi+lo m-matmul
    # deep pools only fit when the nobias fast path frees SBUF (bf16 xn
    # tiles are half the size; bias-correction tensors absent)
    xn_bufs = 8 if (mode == "f32r" and nobias) else 4
    xtg_bufs = 3 if nobias else 2
    ksq_bufs = 3 if nobias else 2

    with tile.TileContext(nc) as tc:
        with (
            tc.tile_pool(name="const", bufs=1) as constp,
            tc.tile_pool(name="wkp", bufs=1) as wkp,
            tc.tile_pool(name="bigw", bufs=2) as bigwp,
            tc.tile_pool(name="xtg", bufs=xtg_bufs) as xtgp,
            tc.tile_pool(name="xnp", bufs=xn_bufs) as xnp,
            tc.tile_pool(name="ksqp", bufs=ksq_bufs) as ksqp,
            tc.tile_pool(name="gate", bufs=2) as gatep,
            tc.tile_pool(name="probs", bufs=3) as probsp,
            tc.tile_pool(name="anoise", bufs=2) as ap_pool,
            tc.tile_pool(name="psum", bufs=2, space="PSUM") as pp,
            tc.tile_pool(name="psum_small", bufs=4, space="PSUM") as pps,
        ):
            # ---------------- constants ----------------
            ident = constp.tile([128, 128], F32, tag="ident")
            make_identity(nc, ident[:])
            ones = constp.tile([128, 1], F32, tag="ones")
            nc.vector.memset(ones[:], 1.0)
            if mode == "hilo":
                ones16 = constp.tile([128, 1], BF16, tag="ones16")
                nc.vector.memset(ones16[:], 1.0)

            def load_bcast(name, ap_in, parts, free):
                t = constp.tile([parts, free], F32, tag=name)
                nc.sync.dma_start(t[:], ap_in.to_broadcast((parts, free)))
                return t

            # q-stage critical-path loads first: inT, bq (the heavy wk load
            # and the remaining broadcasts are queued after the q weights).
            inT_sb = constp.tile([128, CI, BL], F32, tag="inT")
            nc.sync.dma_start(
                inT_sb[:], inT.rearrange("(ci p) b -> p ci b", p=128)
            )
            bq8 = load_bcast("bq8", bq, BL, HID)
            bk8 = None if nobias else load_bcast("bk8", bk, BL, HID)

            wbk_sb = constp.tile([128, CI, NH], F32, tag="wbk")
            nc.sync.dma_start(wbk_sb[:], wbk.rearrange("(ci p) h -> p ci h", p=128))
            def load_wk():
                if mode == "hilo":
                    a = wkp.tile([128, CI, HID], BF16, tag="wk_h", name="wk_sb_h")
                    nc.sync.dma_start(
                        a[:], wk_h.rearrange("(ci p) f -> p ci f", p=128)
                    )
                    b_ = wkp.tile([128, CI, HID], BF16, tag="wk_l", name="wk_sb_l")
                    nc.sync.dma_start(
                        b_[:], wk_l.rearrange("(ci p) f -> p ci f", p=128)
                    )
                    return None, a, b_
                t = wkp.tile([128, CI, HID], xdt, tag="wk", name="wk_sb")
                nc.sync.dma_start(t[:], wk.rearrange("(ci p) f -> p ci f", p=128))
                return t, None, None

            # ---------------- q projection (all local b at once) -------------
            # q[b, f] = input[b] @ Wq + bq ; normalize per head; build the
            # block-diagonal qhat matrix feeding the s_raw psum columns.
            pq = pp.tile([128, 1024], F32, tag="big")
            for qf in range(8):
                wq_sb = bigwp.tile([128, CI, 128], F32, tag="bigw")
                nc.sync.dma_start(
                    wq_sb[:],
                    wq.rearrange("(ci p) f -> p ci f", p=128)[
                        :, :, qf * 128 : (qf + 1) * 128
                    ],
                )
                for ci in range(CI):
                    nc.tensor.matmul(
                        pq[0:BL, qf * 128 : (qf + 1) * 128],
                        inT_sb[:, ci, :],
                        wq_sb[:, ci, :],
                        start=(ci == 0),
                        stop=(ci == CI - 1),
                    )
            wk_sb, wk_sb_h, wk_sb_l = load_wk()
            bv8 = load_bcast("bv8", bv, BL, HID)
            bd8 = load_bcast("bd8", bd, BL, HID)
            sb_sb = None if nobias else load_bcast("sb_sb", sbrep, 128, NH * JC)
            q_sb = constp.tile([BL, HID], F32, tag="q_sb")
            nc.vector.tensor_add(q_sb[:], pq[0:BL, :], bq8[:])
            tmp8 = constp.tile([BL, HID], F32, tag="tmp8")
            nc.scalar.activation(tmp8[:], q_sb[:], AF.Square)
            qssq = constp.tile([BL, NH], F32, tag="qssq")
            nc.vector.reduce_sum(
                qssq[:], tmp8[:].rearrange("b (h d) -> b h d", d=HD), axis=AX.X
            )
            # rqn = 1/sqrt(qssq), one Newton refinement
            rq = constp.tile([BL, NH], F32, tag="rq")
            nc.scalar.activation(rq[:], qssq[:], AF.Sqrt)
            nc.vector.reciprocal(rq[:], rq[:])
            tq = constp.tile([BL, NH], F32, tag="tq")
            nc.vector.tensor_mul(tq[:], rq[:], rq[:])
            nc.vector.tensor_mul(tq[:], tq[:], qssq[:])
            nc.vector.tensor_scalar(tq[:], tq[:], -0.5, 1.5, OP.mult, OP.add)
            nc.vector.tensor_mul(rq[:], rq[:], tq[:])
            # qhat = q * rqn (broadcast rqn over head dim)
            qn = constp.tile([BL, HID], F32, tag="qn")
            nc.vector.tensor_mul(
                qn[:].rearrange("b (h d) -> b h d", d=HD),
                q_sb[:].rearrange("b (h d) -> b h d", d=HD),
                rq[:].unsqueeze(2).to_broadcast([BL, NH, HD]),
            )
            cqn_bc = None
            if not nobias:
                # cqn[b,h] = dot(bk_h, qhat_h)
                nc.vector.tensor_mul(tmp8[:], qn[:], bk8[:])
                cqn = constp.tile([BL, NH], F32, tag="cqn")
                nc.vector.reduce_sum(
                    cqn[:], tmp8[:].rearrange("b (h d) -> b h d", d=HD), axis=AX.X
                )
                # broadcast each b's cqn row across partitions (via DRAM bounce)
                with tc.tile_pool(name="dramtmp", bufs=1, space="DRAM") as dramp:
                    cqn_dram = dramp.tile([BL, NH], F32, tag="cqn_dram")
                    nc.sync.dma_start(cqn_dram[:], cqn[:])
                    cqn_bc = []
                    for b in range(BL):
                        t = constp.tile([128, NH], F32, tag=f"cqn_bc{b}")
                        nc.sync.dma_start(
                            t[:], cqn_dram[b : b + 1, :].to_broadcast((128, NH))
                        )
                        cqn_bc.append(t)

            # transpose qhat -> [c, b] and assemble block-diagonal Qblk
            qnT = constp.tile([128, CI, BL], F32, tag="qnT")
            for ci in range(CI):
                ptr = pps.tile([128, 128], F32, tag="small")
                nc.tensor.transpose(
                    ptr[:, 0:BL],
                    qn[:, ci * 128 : (ci + 1) * 128],
                    ident[0:BL, 0:BL],
                )
                nc.vector.tensor_copy(qnT[:, ci, :], ptr[:, 0:BL])
            # qnblk[f, fi, b, h] = block-diagonal qhat (rows = Wk output feats)
            qnblk = constp.tile([128, CI, BL, NH], F32, tag="qnblk")
            nc.vector.memset(qnblk[:], 0.0)
            for fi in range(CI):
                for half in range(2):
                    h = 2 * fi + half
                    nc.vector.tensor_copy(
                        qnblk[
                            half * 64 : (half + 1) * 64,
                            fi : fi + 1,
                            :,
                            h : h + 1,
                        ],
                        qnT[
                            half * 64 : (half + 1) * 64, fi : fi + 1, :
                        ].unsqueeze(3),
                    )
            # wq_eff[c, (b,h)] = sum_f Wk[c,f] * qnblk[f, (b,h)] via PE with WkT,
            # packed together with wbk into the s-matmul moving operand swblk.
            swblk = constp.tile([128, CI, BL, SW], xdt, tag="swblk")
            for half in range(2):
                wkT_sb = bigwp.tile([128, CI, 512], F32, tag="bigw", name="wkT_sb")
                nc.sync.dma_start(
                    wkT_sb[:],
                    wkT.rearrange("(fi p) c -> p fi c", p=128)[
                        :, :, half * 512 : (half + 1) * 512
                    ],
                )
                for cc in range(4):
                    ci = half * 4 + cc
                    pwq = pps.tile([128, 128], F32, tag="small")
                    for fi in range(CI):
                        nc.tensor.matmul(
                            pwq[:, :],
                            wkT_sb[:, fi, cc * 128 : (cc + 1) * 128],
                            qnblk[:, fi, :, :],
                            start=(fi == 0),
                            stop=(fi == CI - 1),
                        )
                    nc.vector.tensor_copy(
                        swblk[:, ci : ci + 1, :, 0:16],
                        pwq[:].rearrange("p (b h) -> p b h", h=NH).unsqueeze(1),
                    )
            if not nobias:
                for ci in range(CI):
                    nc.vector.tensor_copy(
                        swblk[:, ci : ci + 1, :, 16:32],
                        wbk_sb[:, ci : ci + 1, :]
                        .unsqueeze(2)
                        .to_broadcast([128, 1, BL, NH]),
                    )
            if mode == "hilo":
                swblk_h = constp.tile([128, CI, BL, SW], BF16, tag="swblk_h")
                nc.vector.tensor_copy(swblk_h[:], swblk[:])
                swblk_l = constp.tile([128, CI, BL, SW], BF16, tag="swblk_l")
                nc.vector.tensor_sub(swblk_l[:], swblk[:], swblk_h[:])

            # m-matmul for one b: m[b] = probs[b].T @ x[b]; each b's
            # [NH, HID] block lands at a 32-aligned partition slot.
            m_tiles = [
                constp.tile([128, HID], F32, tag="m_allA", name="m_allA"),
                constp.tile([128, HID], F32, tag="m_allB", name="m_allB"),
            ]
            psp = None if nobias else pps.tile([128, 128], F32, tag="small")

            def emit_m(b):
                pm = pp.tile([128, 1024], F32, tag="big", name="pm")
                for jc in range(JC):
                    if mhilo:
                        xnt_h = xnp.tile([128, HID], BF16, tag="xn_h")
                        nc.sync.dma_start(
                            xnt_h[:], xn_h[b, jc * 128 : (jc + 1) * 128, :]
                        )
                        xnt_l = xnp.tile([128, HID], BF16, tag="xn_l")
                        nc.sync.dma_start(
                            xnt_l[:], xn_l[b, jc * 128 : (jc + 1) * 128, :]
                        )
                        ph, pl = probs_all[b]
                        st = jc == 0
                        sp = jc == JC - 1
                        for bank in range(2):
                            fs = slice(bank * 512, (bank + 1) * 512)
                            nc.tensor.matmul(
                                pm[0:NH, fs], ph[:, jc, :], xnt_h[:, fs],
                                start=st, stop=False,
                            )
                            nc.tensor.matmul(
                                pm[0:NH, fs], ph[:, jc, :], xnt_l[:, fs],
                                start=False, stop=False,
                            )
                            nc.tensor.matmul(
                                pm[0:NH, fs], pl[:, jc, :], xnt_h[:, fs],
                                start=False, stop=sp,
                            )
                        if not nobias:
                            # sp via exact hi+lo accumulation (one psum group)
                            nc.tensor.matmul(
                                psp[0:NH, b : b + 1], ph[:, jc, :], ones16[:],
                                start=(jc == 0), stop=False,
                            )
                            nc.tensor.matmul(
                                psp[0:NH, b : b + 1], pl[:, jc, :], ones16[:],
                                start=False, stop=(jc == JC - 1),
                            )
                    else:
                        xnt = xnp.tile([128, HID], F32, tag="xn")
                        nc.sync.dma_start(
                            xnt[:], xn[b, jc * 128 : (jc + 1) * 128, :]
                        )
                        probs = probs_all[b]
                        for bank in range(2):
                            fs = slice(bank * 512, (bank + 1) * 512)
                            nc.tensor.matmul(
                                pm[0:NH, fs],
                                probs[:, jc, :],
                                xnt[:, fs],
                                start=(jc == 0),
                                stop=(jc == JC - 1),
                            )
                        if not nobias:
                            # sp[b,h] = sum_j probs
                            nc.tensor.matmul(
                                psp[0:NH, b : b + 1],
                                probs[:, jc, :],
                                ones[:],
                                start=(jc == 0),
                                stop=(jc == JC - 1),
                            )
                slot = (b % 4) * 32
                nc.vector.tensor_copy(
                    m_tiles[b // 4][slot : slot + NH, :], pm[0:NH, :]
                )


            # ---------------- k projection + gate, per local batch ----------
            probs_all = []
            for b in range(BL):
                ssq_all = gatep.tile([128, JC, NH], F32, tag="ssq")
                sk_all = gatep.tile([128, JC, SW], F32, tag="sk")
                for jg in range(JC // JG):
                    if mode == "hilo":
                        xg_h = xtgp.tile([128, CI, JG * 128], BF16, tag="xg_h")
                        nc.sync.dma_start(
                            xg_h[:],
                            xt_h[b].rearrange("(ci p) j -> p ci j", p=128)[
                                :, :, jg * JG * 128 : (jg + 1) * JG * 128
                            ],
                        )
                        xg_l = xtgp.tile([128, CI, JG * 128], BF16, tag="xg_l")
                        nc.sync.dma_start(
                            xg_l[:],
                            xt_l[b].rearrange("(ci p) j -> p ci j", p=128)[
                                :, :, jg * JG * 128 : (jg + 1) * JG * 128
                            ],
                        )
                    else:
                        xg = xtgp.tile([128, CI, JG * 128], xdt, tag="xg")
                        nc.sync.dma_start(
                            xg[:],
                            xt[b].rearrange("(ci p) j -> p ci j", p=128)[
                                :, :, jg * JG * 128 : (jg + 1) * JG * 128
                            ],
                        )
                    for jl in range(JG):
                        jc = jg * JG + jl
                        jsl = slice(jl * 128, (jl + 1) * 128)
                        pk = pp.tile([128, 1024], F32, tag="big")
                        ps = pps.tile([128, 128], F32, tag="small")
                        for ci in range(CI):
                            st = ci == 0
                            sp = ci == CI - 1
                            if mode == "hilo":
                                lh = xg_h[:, ci, jsl]
                                ll = xg_l[:, ci, jsl]
                                for bank in range(2):
                                    fs = slice(bank * 512, (bank + 1) * 512)
                                    nc.tensor.matmul(
                                        pk[:, fs], lh, wk_sb_h[:, ci, fs],
                                        start=st, stop=False,
                                    )
                                    nc.tensor.matmul(
                                        pk[:, fs], lh, wk_sb_l[:, ci, fs],
                                        start=False, stop=False,
                                    )
                                    nc.tensor.matmul(
                                        pk[:, fs], ll, wk_sb_h[:, ci, fs],
                                        start=False, stop=sp,
                                    )
                                nc.tensor.matmul(
                                    ps[:, 0:SW], lh, swblk_h[:, ci, b, :],
                                    start=st, stop=False,
                                )
                                nc.tensor.matmul(
                                    ps[:, 0:SW], lh, swblk_l[:, ci, b, :],
                                    start=False, stop=False,
                                )
                                nc.tensor.matmul(
                                    ps[:, 0:SW], ll, swblk_h[:, ci, b, :],
                                    start=False, stop=sp,
                                )
                            else:
                                lhs = xg[:, ci, jsl]
                                for bank in range(2):
                                    fs = slice(bank * 512, (bank + 1) * 512)
                                    nc.tensor.matmul(
                                        pk[:, fs],
                                        lhs,
                                        wk_sb[:, ci, fs],
                                        start=st,
                                        stop=sp,
                                    )
                                nc.tensor.matmul(
                                    ps[:, 0:SW],
                                    lhs,
                                    swblk[:, ci, b, :],
                                    start=st,
                                    stop=sp,
                                )
                        # evictions: k^2 via ACT square; segmented reduce on DVE
                        ksq = ksqp.tile([128, HID], F32, tag="ksq")
                        nc.scalar.activation(ksq[:], pk[:, :], AF.Square)
                        nc.vector.reduce_sum(
                            ssq_all[:, jc, :],
                            ksq[:].rearrange("p (h d) -> p h d", d=HD),
                            axis=AX.X,
                        )
                        nc.vector.tensor_copy(sk_all[:, jc, :], ps[:, 0:SW])

                # ---------------- gate (rational gumbel softmax) ----------
                a0_t = ap_pool.tile([128, JC, NH], F32, tag="a0")
                nc.sync.dma_start(
                    a0_t[:], u_a0[b].rearrange("(jc p) h -> p jc h", p=128)
                )
                a1_t = ap_pool.tile([128, JC, NH], F32, tag="a1")
                nc.sync.dma_start(
                    a1_t[:], u_a1[b].rearrange("(jc p) h -> p jc h", p=128)
                )

                # buffer-reusing gate math: g1..g3 are scratch [128, JC, NH]
                g1 = gatep.tile([128, JC, NH], F32, tag="g1")  # ssq2 -> qt -> den
                g2 = gatep.tile([128, JC, NH], F32, tag="g2")  # r -> num
                g3 = gatep.tile([128, JC, NH], F32, tag="g3")  # newton tmp / sc / rd
                if nobias:
                    g1 = ssq_all  # ||k||^2 needs no bias correction
                else:
                    nc.vector.scalar_tensor_tensor(
                        g1[:], sk_all[:, :, 16:32], 2.0, ssq_all[:], OP.mult, OP.add
                    )
                    nc.vector.tensor_add(
                        g1[:], g1[:], sb_sb[:].rearrange("p (jc h) -> p jc h", h=NH)
                    )
                # g2 = rsqrt(g1) with one Newton step
                nc.scalar.activation(g2[:], g1[:], AF.Sqrt)
                nc.vector.reciprocal(g2[:], g2[:])
                nc.vector.tensor_mul(g3[:], g2[:], g2[:])
                nc.vector.tensor_mul(g3[:], g3[:], g1[:])
                nc.vector.tensor_scalar(g3[:], g3[:], -0.5, 1.5, OP.mult, OP.add)
                nc.vector.tensor_mul(g2[:], g2[:], g3[:])
                # g3 = scores = (s_raw + cqn) * rsqrt
                if nobias:
                    nc.vector.tensor_mul(g3[:], sk_all[:, :, 0:16], g2[:])
                else:
                    nc.vector.tensor_add(
                        g3[:],
                        sk_all[:, :, 0:16],
                        cqn_bc[b][:].unsqueeze(1).to_broadcast([128, JC, NH]),
                    )
                    nc.vector.tensor_mul(g3[:], g3[:], g2[:])
                # p = (scores+1)/2 ; num = p*A1 ; den = num + (1-p)*A0
                nc.vector.tensor_scalar(g2[:], g3[:], 0.5, 0.5, OP.mult, OP.add)
                nc.vector.tensor_scalar(g1[:], g3[:], -0.5, 0.5, OP.mult, OP.add)
                nc.vector.tensor_mul(g2[:], g2[:], a1_t[:])  # num
                nc.vector.tensor_mul(g1[:], g1[:], a0_t[:])
                nc.vector.tensor_add(g1[:], g1[:], g2[:])  # den
                # probs = num * refined_recip(den)
                nc.vector.reciprocal(g3[:], g1[:])
                nc.vector.tensor_mul(g1[:], g1[:], g3[:])
                nc.vector.tensor_scalar(g1[:], g1[:], -1.0, 2.0, OP.mult, OP.add)
                nc.vector.tensor_mul(g3[:], g3[:], g1[:])
                if mhilo:
                    probs = gatep.tile([128, JC, NH], F32, tag="probs_t")
                    nc.vector.tensor_mul(probs[:], g2[:], g3[:])
                    ph = probsp.tile([128, JC, NH], BF16, tag="probs_h")
                    nc.vector.tensor_copy(ph[:], probs[:])
                    pl = probsp.tile([128, JC, NH], BF16, tag="probs_l")
                    nc.vector.tensor_sub(pl[:], probs[:], ph[:])
                    probs_all.append((ph, pl))
                else:
                    probs = probsp.tile([128, JC, NH], F32, tag="probs")
                    nc.vector.tensor_mul(probs[:], g2[:], g3[:])
                    probs_all.append(probs)

                # interleave m(b-1) behind this b's k-projection: its (DMA
                # heavy, PE light) work rides the spare bandwidth, and
                # gate(b-1) had a full kproj round to finish.
                if b >= 1:
                    emit_m(b - 1)

            # ---------------- m-matmul epilogue: last b ----------------------
            emit_m(BL - 1)

            # ---------------- ctx + final dense ------------------------------
            # transpose m -> mT[c, (b,h)]
            mT = constp.tile([128, CI, 128], F32, tag="mT")
            for ci in range(CI):
                for b in range(BL):
                    slot = (b % 4) * 32
                    ptr = pps.tile([128, 128], F32, tag="small")
                    nc.tensor.transpose(
                        ptr[:, 0:NH],
                        m_tiles[b // 4][
                            slot : slot + NH, ci * 128 : (ci + 1) * 128
                        ],
                        ident[slot : slot + NH, slot : slot + NH],
                        tile_position=(slot, 0),
                    )
                    nc.vector.tensor_copy(
                        mT[:, ci : ci + 1, b * NH : (b + 1) * NH],
                        ptr[:, 0:NH].unsqueeze(1),
                    )
            if not nobias:
                # sp: psum [NH, BL] -> sbuf -> transpose -> [BL, NH]
                spT = constp.tile([NH, BL], F32, tag="spT")
                nc.vector.tensor_copy(spT[:], psp[0:NH, 0:BL])
                psp2 = pps.tile([128, 128], F32, tag="small")
                nc.tensor.transpose(psp2[0:BL, 0:NH], spT[:], ident[0:NH, 0:NH])
                sp_all = constp.tile([BL, NH], F32, tag="sp_all")
                nc.vector.tensor_copy(sp_all[:], psp2[0:BL, 0:NH])

            # ctx[b, (h,d)] = sum_ci mT[:, ci, (b,h)] @ Wv[ci, (h,d)]
            pctx = pp.tile([128, 1024], F32, tag="big")
            for hf in range(2):
                wv_sb = bigwp.tile([128, CI, 512], F32, tag="bigw")
                nc.sync.dma_start(
                    wv_sb[:],
                    wv.rearrange("(ci p) f -> p ci f", p=128)[
                        :, :, hf * 512 : (hf + 1) * 512
                    ],
                )
                for hh in range(NH // 2):
                    h = hf * (NH // 2) + hh
                    for ci in range(CI):
                        nc.tensor.matmul(
                            pctx[0:BL, h * HD : (h + 1) * HD],
                            mT[:, ci, h : 128 : NH],
                            wv_sb[:, ci, hh * HD : (hh + 1) * HD],
                            start=(ci == 0),
                            stop=(ci == CI - 1),
                        )
            # ctx += sp * bv (broadcast over d)
            ctx_sb = constp.tile([BL, HID], F32, tag="ctx")
            if nobias:
                nc.vector.tensor_copy(ctx_sb[:], pctx[0:BL, :])
            else:
                tbv = constp.tile([BL, HID], F32, tag="tbv")
                nc.vector.tensor_mul(
                    tbv[:].rearrange("b (h d) -> b h d", d=HD),
                    bv8[:].rearrange("b (h d) -> b h d", d=HD),
                    sp_all[:].unsqueeze(2).to_broadcast([BL, NH, HD]),
                )
                nc.vector.tensor_add(ctx_sb[:], pctx[0:BL, :], tbv[:])
            # transpose ctx -> [c, b]
            ctxT = constp.tile([128, CI, BL], F32, tag="ctxT")
            for ci in range(CI):
                ptr = pps.tile([128, 128], F32, tag="small")
                nc.tensor.transpose(
                    ptr[:, 0:BL],
                    ctx_sb[:, ci * 128 : (ci + 1) * 128],
                    ident[0:BL, 0:BL],
                )
                nc.vector.tensor_copy(ctxT[:, ci, :], ptr[:, 0:BL])
            # out = ctx @ Wd + bd
            po = pp.tile([128, 1024], F32, tag="big")
            for hf in range(2):
                wd_sb = bigwp.tile([128, CI, 512], F32, tag="bigw")
                nc.sync.dma_start(
                    wd_sb[:],
                    wd.rearrange("(ci p) f -> p ci f", p=128)[
                        :, :, hf * 512 : (hf + 1) * 512
                    ],
                )
                for ci in range(CI):
                    nc.tensor.matmul(
                        po[0:BL, hf * 512 : (hf + 1) * 512],
                        ctxT[:, ci, :],
                        wd_sb[:, ci, :],
                        start=(ci == 0),
                        stop=(ci == CI - 1),
                    )
            o_sb = constp.tile([BL, HID], F32, tag="o_sb")
            nc.vector.tensor_add(o_sb[:], po[0:BL, :], bd8[:])
            nc.sync.dma_start(out[:], o_sb[:])

    nc.compile()
    return nc


def _split_hilo(x):
    import ml_dtypes

    h = x.astype(ml_dtypes.bfloat16)
    l = (x - h.astype(np.float32)).astype(ml_dtypes.bfloat16)
    return h, l


def prep_in_maps(inputs, mode=MM_MODE):
    """Host-side staging: shard over batch, transpose/relayout, noise logs."""
    it = np.asarray(inputs["input_tensor"], dtype=np.float32)  # (B, 1, HID)
    rt = np.asarray(inputs["retrieval_tensor"], dtype=np.float32)  # (B, SK, HID)
    un = np.asarray(inputs["u_noise"], dtype=np.float32)  # (B, NH, 1, SK, 2)
    wq = np.asarray(inputs["Wq"], dtype=np.float32)
    wk = np.asarray(inputs["Wk"], dtype=np.float32)
    wv = np.asarray(inputs["Wv"], dtype=np.float32)
    wd = np.asarray(inputs["Wd"], dtype=np.float32)
    bq = np.asarray(inputs["bq"], dtype=np.float32).reshape(1, HID)
    bk = np.asarray(inputs["bk"], dtype=np.float32).reshape(1, HID)
    bv = np.asarray(inputs["bv"], dtype=np.float32).reshape(1, HID)
    bd = np.asarray(inputs["bd"], dtype=np.float32).reshape(1, HID)

    bk_heads = bk.reshape(NH, HD)
    wbk = np.einsum(
        "chd,hd->ch", wk.reshape(HID, NH, HD), bk_heads
    ).astype(np.float32)  # (HID, NH)
    sb = np.tile((bk_heads**2).sum(axis=1), JC).reshape(1, NH * JC).astype(np.float32)

    # A_i = EPS - log(u_i + EPS), computed in fp32 like the reference
    u0 = un[:, :, 0, :, 0].transpose(0, 2, 1)  # (B, SK, NH)
    u1 = un[:, :, 0, :, 1].transpose(0, 2, 1)
    a0 = (np.float32(EPS) - np.log(u0 + np.float32(EPS), dtype=np.float32)).astype(
        np.float32
    )
    a1 = (np.float32(EPS) - np.log(u1 + np.float32(EPS), dtype=np.float32)).astype(
        np.float32
    )

    shared = {
        "wq": wq, "wv": wv, "wd": wd,
        "bq": bq, "bk": bk, "bv": bv, "bd": bd,
        "sbrep": sb,
    }
    shared["wbk"] = wbk
    shared["wkT"] = np.ascontiguousarray(wk.T)
    if mode == "hilo":
        wk_h, wk_l = _split_hilo(wk)
        shared.update(wk_h=wk_h, wk_l=wk_l)
    else:
        shared.update(wk=wk)

    in_maps = []
    for c in range(NCORES):
        bs = slice(c * BL, (c + 1) * BL)
        xn_c = np.ascontiguousarray(rt[bs])
        xt_c = np.ascontiguousarray(rt[bs].transpose(0, 2, 1))
        m = {
            "xn": xn_c,
            "a0": np.ascontiguousarray(a0[bs]),
            "a1": np.ascontiguousarray(a1[bs]),
            "inT": np.ascontiguousarray(it[bs, 0, :].T),
            **shared,
        }
        if mode == "hilo":
            m["xt_h"], m["xt_l"] = _split_hilo(xt_c)
            m["xn_h"], m["xn_l"] = _split_hilo(xn_c)
            del m["xn"]
        else:
            m["xt"] = xt_c
            if mode == "f32r":
                # nobias builds take the bf16 hi+lo m-matmul inputs; the
                # runner only transfers tensors the NEFF declares.
                m["xn_h"], m["xn_l"] = _split_hilo(xn_c)
        in_maps.append(m)
    return in_maps


_NC_CACHE = {}


def kernel(**inputs) -> np.ndarray:
    mode = MM_MODE
    nobias = all(
        not np.any(np.asarray(inputs[k])) for k in ("bq", "bk", "bv", "bd")
    )
    key = (mode, nobias)
    if key not in _NC_CACHE:
        _NC_CACHE[key] = build_nc(mode, nobias)
    nc = _NC_CACHE[key]
    in_maps = prep_in_maps(inputs, mode)
    res = run_bass_kernel_spmd(nc, in_maps, core_ids=list(range(NCORES)))
    return np.concatenate([res.results[c]["o"] for c in range(NCORES)], axis=0)



# revision 7
# speedup vs baseline: 1.3444x; 1.3444x over previous
"""Trainium2 Bass kernel for nn_AttentionBasedMerger.

Reference computation (per batch element b, SQ=1):
  q = input @ Wq + bq                      -> (NH, HD)  [tiny]
  k = retrieval @ Wk + bk                  -> (SK, NH, HD)
  v = retrieval @ Wv + bv                  -> (SK, NH, HD)
  scores[h,j] = cos_sim(q[h], k[j,h])
  p = (scores+1)/2 ; 2-way gumbel-softmax gate with external uniform noise
  probs[h,j] = gate[...,0]
  ctx[h] = sum_j probs[h,j] v[j,h]         -> (NH, HD)
  out = ctx.flat @ Wd + bd                 -> (HID,)

Measured bottleneck of the whole pipeline is host->device input bytes, so the
kernel is organised around shipping the retrieval tensor exactly ONCE, in
bf16, natural [SK, HID] layout (32MB/core), and deriving every other layout
on-device:
  - k-projection needs x^T (contraction dim on partitions): each 128x128
    subtile is transposed on the PE (bf16 transpose, 1 cyc/row).
  - m-matmul (probs^T @ x) uses the natural tiles directly as the stationary
    operand, which also produces m already transposed for the ctx GEMM.

Algebraic restructuring (exact up to fp reassociation):
  - v-projection is never materialized:
      ctx[h] = (sum_j probs[h,j] x[j]) @ Wv_h (+ (sum_j probs[h,j]) * bv_h)
  - scores come from the k-projection run once:
      s_raw[j,(b,h)] = x[j] @ swblk        (16 extra psum columns;
                                            swblk = Wk @ qhat_blockdiag,
                                            computed on host: q-projection +
                                            normalize is 0.05% of FLOPs)
      ||k||^2        = sum_d k^2           (squared-eviction + reduce)
      scores         = s_raw * rsqrt(||k||^2)
  - the 2-way gumbel softmax collapses to a stable rational:
      probs = p*A1 / (p*A1 + (1-p)*A0),  A_i = EPS - log(u_i + EPS)
    (A0/A1 computed on host, shipped as fp16: floating point keeps the
    relative error bounded even for A -> 0.)

Numerics: bf16 operands everywhere with f32 PSUM accumulation; emulated
end-to-end rel err 4.1e-3 vs the 2e-2 gate.

Sharding: pure data-parallel over batch, 8 batch elements per core.
bq is folded into the host-side qhat; the device bias path (bk/bv/bd) exists
but is not exercised by the reference (all biases are zero).
"""

import sys

sys.path.insert(0, "/opt/trn_rl_repo")

import ml_dtypes
import numpy as np

import concourse.bass as bass  # noqa: F401  (import keeps bass registered)
import concourse.tile as tile
from concourse import bacc, mybir
from concourse.bass_utils import run_bass_kernel_spmd
from concourse.masks import make_identity

F32 = mybir.dt.float32
BF16 = mybir.dt.bfloat16
F16 = mybir.dt.float16
AX = mybir.AxisListType
OP = mybir.AluOpType
AF = mybir.ActivationFunctionType

B, SK, HID, NH, HD = 64, 2048, 1024, 16, 64
NCORES = 8
BL = B // NCORES  # 8 batch elems per core
CI = HID // 128  # 8 contraction chunks
JC = SK // 128  # 16 seq chunks
EPS = 1e-20


def build_nc(nobias=True):
    nc = bacc.Bacc("TRN2", target_bir_lowering=False, debug=False, num_devices=NCORES)

    def din(name, shape, dt):
        return nc.dram_tensor(name, list(shape), dt, kind="ExternalInput").ap()

    x = din("x", [BL, SK, HID], BF16)
    a0 = din("a0", [BL, SK, NH], F16)
    a1 = din("a1", [BL, SK, NH], F16)
    wk = din("wk", [HID, HID], BF16)
    wv = din("wv", [HID, HID], BF16)
    wd = din("wd", [HID, HID], BF16)
    swblk_d = din("swblk", [128, CI, BL, NH], BF16)
    if not nobias:
        wbk_d = din("wbk", [128, CI, NH], BF16)
        cqn_d = din("cqn", [BL, NH], F32)
        sbrep_d = din("sbrep", [1, NH * JC], F32)
        bv_d = din("bv", [1, HID], F32)
        bd_d = din("bd", [1, HID], F32)
    out = nc.dram_tensor("o", [BL, HID], F32, kind="ExternalOutput").ap()

    SW = NH if nobias else 2 * NH

    with tile.TileContext(nc) as tc:
        with (
            tc.tile_pool(name="const", bufs=1) as constp,
            tc.tile_pool(name="wkp", bufs=1) as wkp,
            tc.tile_pool(name="bigw", bufs=2) as bigwp,
            tc.tile_pool(name="xn", bufs=20) as xnp,
            tc.tile_pool(name="xt", bufs=3) as xtp,
            tc.tile_pool(name="ksq", bufs=3) as ksqp,
            tc.tile_pool(name="gate", bufs=2) as gatep,
            tc.tile_pool(name="noise", bufs=2) as ap_pool,
            tc.tile_pool(name="probsp", bufs=3) as probsp,
            tc.tile_pool(name="pbig", bufs=2, space="PSUM") as pp,
            tc.tile_pool(name="ptrp", bufs=2, space="PSUM") as ptrp,
            tc.tile_pool(name="psml", bufs=2 if nobias else 1, space="PSUM") as pps,
        ):
            identB = constp.tile([128, 128], BF16, tag="identB")
            make_identity(nc, identB[:])

            wk_sb = wkp.tile([128, CI, HID], BF16, tag="wk")
            nc.sync.dma_start(wk_sb[:], wk.rearrange("(ci p) f -> p ci f", p=128))
            swblk = constp.tile([128, CI, BL, NH], BF16, tag="swblk")
            nc.sync.dma_start(swblk[:], swblk_d)
            # epilogue weights: queue the DMA early so it rides spare bandwidth
            wv_sb = bigwp.tile([128, CI, HID], BF16, tag="bigw", name="wv_sb")
            nc.sync.dma_start(wv_sb[:], wv.rearrange("(ci p) f -> p ci f", p=128))
            wd_sb = bigwp.tile([128, CI, HID], BF16, tag="bigw", name="wd_sb")
            nc.sync.dma_start(wd_sb[:], wd.rearrange("(ci p) f -> p ci f", p=128))

            if not nobias:
                wbk_sb = constp.tile([128, CI, NH], BF16, tag="wbk")
                nc.sync.dma_start(wbk_sb[:], wbk_d)
                sb_sb = constp.tile([128, NH * JC], F32, tag="sb_sb")
                nc.sync.dma_start(sb_sb[:], sbrep_d.to_broadcast((128, NH * JC)))
                bv8 = constp.tile([BL, HID], F32, tag="bv8")
                nc.sync.dma_start(bv8[:], bv_d.to_broadcast((BL, HID)))
                bd8 = constp.tile([BL, HID], F32, tag="bd8")
                nc.sync.dma_start(bd8[:], bd_d.to_broadcast((BL, HID)))
                cqn_bc = []
                for b in range(BL):
                    t = constp.tile([128, NH], F32, tag=f"cqn{b}", name=f"cqn{b}")
                    nc.sync.dma_start(
                        t[:], cqn_d[b : b + 1, :].to_broadcast((128, NH))
                    )
                    cqn_bc.append(t)
                ones16 = constp.tile([128, 1], BF16, tag="ones16")
                nc.vector.memset(ones16[:], 1.0)
                psp = pps.tile([128, BL], F32, tag="psp")

            # m output, already transposed: mT[c, ci, b*NH+h]
            mT = constp.tile([128, CI, BL * NH], BF16, tag="mT")

            # ---------------- main loop: per local batch ----------------
            for b in range(BL):
                ssq_all = gatep.tile([128, JC, NH], F32, tag="ssq", name="ssq_all")
                sk_all = gatep.tile([128, JC, SW], F32, tag="sk", name="sk_all")
                xn_tiles = []
                for jc in range(JC):
                    xnt = xnp.tile([128, HID], BF16, tag="xn", name="xnt")
                    nc.sync.dma_start(xnt[:], x[b, jc * 128 : (jc + 1) * 128, :])
                    xn_tiles.append(xnt)
                    # on-device transpose of the 8 subtiles -> x^T for kproj
                    # (all 8 land in one psum bank; one DVE eviction)
                    xtt = xtp.tile([128, CI, 128], BF16, tag="xt", name="xtt")
                    ptr = ptrp.tile([128, CI * 128], BF16, tag="ptr", name="ptr")
                    for cc in range(CI):
                        nc.tensor.transpose(
                            ptr[:, cc * 128 : (cc + 1) * 128],
                            xnt[:, cc * 128 : (cc + 1) * 128],
                            identB[:],
                        )
                    nc.vector.tensor_copy(
                        xtt[:], ptr[:].rearrange("p (ci j) -> p ci j", j=128)
                    )
                    # k-projection + score columns
                    pk = pp.tile([128, HID], F32, tag="pk", name="pk")
                    ps = pps.tile([128, SW], F32, tag="sml", name="ps")
                    for ci in range(CI):
                        st = ci == 0
                        sp_ = ci == CI - 1
                        lhs = xtt[:, ci, :]
                        for bank in range(2):
                            fs = slice(bank * 512, (bank + 1) * 512)
                            nc.tensor.matmul(
                                pk[:, fs], lhs, wk_sb[:, ci, fs], start=st, stop=sp_
                            )
                        nc.tensor.matmul(
                            ps[:, 0:NH], lhs, swblk[:, ci, b, :], start=st, stop=sp_
                        )
                        if not nobias:
                            nc.tensor.matmul(
                                ps[:, NH:SW], lhs, wbk_sb[:, ci, :], start=st, stop=sp_
                            )
                    # evictions: k^2 via ACT square; segmented reduce on DVE
                    ksq = ksqp.tile([128, HID], F32, tag="ksq", name="ksq")
                    nc.scalar.activation(ksq[:], pk[:, :], AF.Square)
                    nc.vector.reduce_sum(
                        ssq_all[:, jc, :],
                        ksq[:].rearrange("p (h d) -> p h d", d=HD),
                        axis=AX.X,
                    )
                    nc.vector.tensor_copy(sk_all[:, jc, :], ps[:, 0:SW])

                # ---------------- gate (rational gumbel softmax) ----------
                a0_t = ap_pool.tile([128, JC, NH], F16, tag="a0", name="a0_t")
                nc.sync.dma_start(
                    a0_t[:], a0[b].rearrange("(jc p) h -> p jc h", p=128)
                )
                a1_t = ap_pool.tile([128, JC, NH], F16, tag="a1", name="a1_t")
                nc.sync.dma_start(
                    a1_t[:], a1[b].rearrange("(jc p) h -> p jc h", p=128)
                )
                a0f = ap_pool.tile([128, JC, NH], F32, tag="a0f", name="a0f")
                nc.vector.tensor_copy(a0f[:], a0_t[:])
                a1f = ap_pool.tile([128, JC, NH], F32, tag="a1f", name="a1f")
                nc.vector.tensor_copy(a1f[:], a1_t[:])

                g1 = gatep.tile([128, JC, NH], F32, tag="g1", name="g1")
                g2 = gatep.tile([128, JC, NH], F32, tag="g2", name="g2")
                g3 = gatep.tile([128, JC, NH], F32, tag="g3", name="g3")
                if nobias:
                    gden = ssq_all  # ||k||^2 needs no bias correction
                else:
                    nc.vector.scalar_tensor_tensor(
                        g1[:], sk_all[:, :, NH:SW], 2.0, ssq_all[:], OP.mult, OP.add
                    )
                    nc.vector.tensor_add(
                        g1[:], g1[:], sb_sb[:].rearrange("p (jc h) -> p jc h", h=NH)
                    )
                    gden = g1
                # g2 = rsqrt(gden) with one Newton step
                nc.scalar.activation(g2[:], gden[:], AF.Sqrt)
                nc.vector.reciprocal(g2[:], g2[:])
                nc.vector.tensor_mul(g3[:], g2[:], g2[:])
                nc.vector.tensor_mul(g3[:], g3[:], gden[:])
                nc.vector.tensor_scalar(g3[:], g3[:], -0.5, 1.5, OP.mult, OP.add)
                nc.vector.tensor_mul(g2[:], g2[:], g3[:])
                # g3 = scores = (s_raw (+ cqn)) * rsqrt
                if nobias:
                    nc.vector.tensor_mul(g3[:], sk_all[:, :, 0:NH], g2[:])
                else:
                    nc.vector.tensor_add(
                        g3[:],
                        sk_all[:, :, 0:NH],
                        cqn_bc[b][:].unsqueeze(1).to_broadcast([128, JC, NH]),
                    )
                    nc.vector.tensor_mul(g3[:], g3[:], g2[:])
                # p = (scores+1)/2 ; num = p*A1 ; den = num + (1-p)*A0
                nc.vector.tensor_scalar(g2[:], g3[:], 0.5, 0.5, OP.mult, OP.add)
                nc.vector.tensor_scalar(g1[:], g3[:], -0.5, 0.5, OP.mult, OP.add)
                nc.vector.tensor_mul(g2[:], g2[:], a1f[:])  # num
                nc.vector.tensor_mul(g1[:], g1[:], a0f[:])
                nc.vector.tensor_add(g1[:], g1[:], g2[:])  # den
                # probs = num * refined_recip(den)
                nc.vector.reciprocal(g3[:], g1[:])
                nc.vector.tensor_mul(g1[:], g1[:], g3[:])
                nc.vector.tensor_scalar(g1[:], g1[:], -1.0, 2.0, OP.mult, OP.add)
                nc.vector.tensor_mul(g3[:], g3[:], g1[:])
                nc.vector.tensor_mul(g2[:], g2[:], g3[:])  # probs (f32)
                probs = probsp.tile([128, JC, NH], BF16, tag="probs", name="probs")
                nc.vector.tensor_copy(probs[:], g2[:])

                # ---------------- m-matmul: m^T[c, h] = sum_j x[j,c] probs[j,h]
                # natural xn tiles as stationary -> output lands pre-transposed
                for cc in range(CI):
                    pm = pps.tile([128, NH], F32, tag="sml", name="pm")
                    for jc in range(JC):
                        nc.tensor.matmul(
                            pm[:, :],
                            xn_tiles[jc][:, cc * 128 : (cc + 1) * 128],
                            probs[:, jc, :],
                            start=(jc == 0),
                            stop=(jc == JC - 1),
                        )
                    nc.vector.tensor_copy(mT[:, cc, b * NH : (b + 1) * NH], pm[:, :])
                if not nobias:
                    # sp[h, b] = sum_j probs
                    for jc in range(JC):
                        nc.tensor.matmul(
                            psp[0:NH, b : b + 1],
                            probs[:, jc, :],
                            ones16[:],
                            start=(jc == 0),
                            stop=(jc == JC - 1),
                        )

            # ---------------- ctx + final dense ------------------------------
            # ctx[b, (h,d)] = sum_ci mT[:, ci, (b,h)] @ Wv[ci, (h,d)]
            pctx = pp.tile([128, HID], F32, tag="pk", name="pctx")
            for h in range(NH):
                for ci in range(CI):
                    nc.tensor.matmul(
                        pctx[0:BL, h * HD : (h + 1) * HD],
                        mT[:, ci, h : BL * NH : NH],
                        wv_sb[:, ci, h * HD : (h + 1) * HD],
                        start=(ci == 0),
                        stop=(ci == CI - 1),
                    )
            ctx_sb = constp.tile([BL, HID], BF16, tag="ctx_sb")
            if nobias:
                nc.vector.tensor_copy(ctx_sb[:], pctx[0:BL, :])
            else:
                # sp: psum [NH, BL] -> transpose -> [BL, NH]
                spT = constp.tile([NH, BL], BF16, tag="spT")
                nc.vector.tensor_copy(spT[:], psp[0:NH, 0:BL])
                ptr_sp = ptrp.tile([128, 128], BF16, tag="ptr", name="ptr_sp")
                nc.tensor.transpose(
                    ptr_sp[0:BL, 0:NH], spT[:], identB[0:NH, 0:NH]
                )
                sp_all = constp.tile([BL, NH], F32, tag="sp_all")
                nc.vector.tensor_copy(sp_all[:], ptr_sp[0:BL, 0:NH])
                ctxf = constp.tile([BL, HID], F32, tag="ctxf")
                nc.vector.tensor_mul(
                    ctxf[:].rearrange("b (h d) -> b h d", d=HD),
                    bv8[:].rearrange("b (h d) -> b h d", d=HD),
                    sp_all[:].unsqueeze(2).to_broadcast([BL, NH, HD]),
                )
                nc.vector.tensor_add(ctxf[:], ctxf[:], pctx[0:BL, :])
                nc.vector.tensor_copy(ctx_sb[:], ctxf[:])
            # transpose ctx -> [c, b]
            ctxT = constp.tile([128, CI, BL], BF16, tag="ctxT")
            for ci in range(CI):
                ptr_c = ptrp.tile([128, 128], BF16, tag="ptr", name="ptr_c")
                nc.tensor.transpose(
                    ptr_c[:, 0:BL],
                    ctx_sb[:, ci * 128 : (ci + 1) * 128],
                    identB[0:BL, 0:BL],
                )
                nc.vector.tensor_copy(ctxT[:, ci, :], ptr_c[:, 0:BL])
            # out = ctx @ Wd (+ bd)
            po = pp.tile([128, HID], F32, tag="pk", name="po")
            for ci in range(CI):
                st = ci == 0
                sp_ = ci == CI - 1
                for bank in range(2):
                    fs = slice(bank * 512, (bank + 1) * 512)
                    nc.tensor.matmul(
                        po[0:BL, fs], ctxT[:, ci, :], wd_sb[:, ci, fs],
                        start=st, stop=sp_,
                    )
            o_sb = constp.tile([BL, HID], F32, tag="o_sb")
            if nobias:
                nc.vector.tensor_copy(o_sb[:], po[0:BL, :])
            else:
                nc.vector.tensor_add(o_sb[:], po[0:BL, :], bd8[:])
            nc.sync.dma_start(out[:], o_sb[:])

    nc.compile()
    return nc


def prep_in_maps(inputs, nobias=True):
    """Host-side staging: batch shard, bf16/fp16 quantization, q-projection
    and gate-noise logs (both tiny compared to the device GEMMs)."""
    it = np.asarray(inputs["input_tensor"], dtype=np.float32)  # (B, 1, HID)
    rt = np.asarray(inputs["retrieval_tensor"], dtype=np.float32)  # (B, SK, HID)
    un = np.asarray(inputs["u_noise"], dtype=np.float32)  # (B, NH, 1, SK, 2)
    Wq = np.asarray(inputs["Wq"], dtype=np.float32)
    Wk = np.asarray(inputs["Wk"], dtype=np.float32)
    Wv = np.asarray(inputs["Wv"], dtype=np.float32)
    Wd = np.asarray(inputs["Wd"], dtype=np.float32)
    bq = np.asarray(inputs["bq"], dtype=np.float32).reshape(HID)

    # qhat on host (f32, matches reference precision); bq folded in here
    q = it[:, 0, :] @ Wq + bq
    qh = q.reshape(B, NH, HD)
    qh = qh / np.linalg.norm(qh, axis=-1, keepdims=True)  # (B, NH, HD)

    # swblk[c, (b,h)] = Wk @ qhat_blockdiag, laid out [p, ci, b, h]
    Wkh = Wk.reshape(HID, NH, HD)
    wq_eff = np.einsum("chd,bhd->cbh", Wkh, qh, optimize=True)  # (HID, B, NH)
    swblk = np.ascontiguousarray(
        wq_eff.reshape(CI, 128, B, NH).transpose(1, 0, 2, 3)
    ).astype(ml_dtypes.bfloat16)  # (128, CI, B, NH)

    # A_i = EPS - log(u_i + EPS); fp16 keeps relative error bounded
    u0 = un[:, :, 0, :, 0].transpose(0, 2, 1)  # (B, SK, NH)
    u1 = un[:, :, 0, :, 1].transpose(0, 2, 1)
    a0 = (np.float32(EPS) - np.log(u0 + np.float32(EPS), dtype=np.float32)).astype(
        np.float16
    )
    a1 = (np.float32(EPS) - np.log(u1 + np.float32(EPS), dtype=np.float32)).astype(
        np.float16
    )

    x_bf = rt.astype(ml_dtypes.bfloat16)  # (B, SK, HID)

    shared = {
        "wk": Wk.astype(ml_dtypes.bfloat16),
        "wv": Wv.astype(ml_dtypes.bfloat16),
        "wd": Wd.astype(ml_dtypes.bfloat16),
    }
    if not nobias:
        bk = np.asarray(inputs["bk"], dtype=np.float32).reshape(NH, HD)
        wbk = np.einsum("chd,hd->ch", Wkh, bk)  # (HID, NH)
        shared["wbk"] = np.ascontiguousarray(
            wbk.reshape(CI, 128, NH).transpose(1, 0, 2)
        ).astype(ml_dtypes.bfloat16)
        shared["sbrep"] = (
            np.tile((bk**2).sum(axis=1), JC).reshape(1, NH * JC).astype(np.float32)
        )
        shared["bv"] = np.asarray(inputs["bv"], dtype=np.float32).reshape(1, HID)
        shared["bd"] = np.asarray(inputs["bd"], dtype=np.float32).reshape(1, HID)
        cqn = np.einsum("bhd,hd->bh", qh, bk).astype(np.float32)  # (B, NH)

    in_maps = []
    for c in range(NCORES):
        bs = slice(c * BL, (c + 1) * BL)
        m = {
            "x": np.ascontiguousarray(x_bf[bs]),
            "a0": np.ascontiguousarray(a0[bs]),
            "a1": np.ascontiguousarray(a1[bs]),
            "swblk": np.ascontiguousarray(swblk[:, :, bs, :]),
            **shared,
        }
        if not nobias:
            m["cqn"] = np.ascontiguousarray(cqn[bs])
        in_maps.append(m)
    return in_maps


_NC_CACHE = {}


def kernel(**inputs) -> np.ndarray:
    # bq is folded into the host-side q-projection, so only bk/bv/bd need the
    # device bias path
    nobias = all(
        not np.any(np.asarray(inputs[k])) for k in ("bk", "bv", "bd")
    )
    if nobias not in _NC_CACHE:
        _NC_CACHE[nobias] = build_nc(nobias)
    nc = _NC_CACHE[nobias]
    in_maps = prep_in_maps(inputs, nobias)
    res = run_bass_kernel_spmd(nc, in_maps, core_ids=list(range(NCORES)))
    return np.concatenate([res.results[c]["o"] for c in range(NCORES)], axis=0)
